# revision 56
# baseline (speedup 1.0000x reference)
"""FAPE loss kernel for Trainium2 (Bass/Tile), 8 NeuronCores.

Problem: B=8, N=1024.  reference computes, per batch b:
    R_i, t_i = backbone frames from (n, ca, c)          [N,3,3],[N,3]
    diff[i,j] = || R_i^T (pred_j - t_i) - R_i^T (true_j - t_i) ||
    per_pair  = min(diff,10) + 0.5*(diff - min(diff,10)) = 0.5*(diff + min(diff,10))
    out = sum_b sum_ij m_i m_j per_pair / (sum(m) + 1e-8)

Key identity: both pred and true are expressed in the SAME frame i, so
    R_i^T (pred_j - t_i) - R_i^T (true_j - t_i) = R_i^T d_j,  d_j = pred_j - true_j
and R_i is orthonormal by construction, hence diff[i,j] = ||d_j||
(independent of i) up to the 1e-8 normalize-eps and f32 rounding
(~6e-7 end-to-end vs the jax reference; tolerance is 2e-2).  The O(N^2)
pairwise reduction factorizes exactly:
    sum_ij m_i m_j f(||d_j||) = (sum_i m_i) * (sum_j m_j f(||d_j||))
leaving O(N) device work per batch.  The mask is folded into the packed
inputs on the host (pred_j, true_j both scaled by m_j => masked j gives
d_j = 0 and f(0) = 0, exactly), so the device computes sum_j f(||d_j||).

Per-core body (one batch per core, j = 8*p + t), 6 instrs + output:
    d    = pred - true                        [128,8,3]  DVE/Pool (D_PAT)
    sq   = d * d                              [128,24]   DVE/Pool (SQ_PAT)
    nsq  = reduce_X(sq)                       [128,8]    DVE
    dm   = Sqrt(nsq)                          [128,8]    ACT
    stt  = (dm min 10) + dm, accum_out=ps_b   [128,8]    DVE  (per-partition
           sum fused into the clamp op's accumulate port -> ps_b [128,1])
    mm   = ones[128,1]^T @ ps_b -> pr[0, b]   [1,1]  PE (partition sum into
           body slot b of a chunk-shared [1,CHUNK] PSUM tile)
Output path (OUT_MODE="block", once per CHUNK=80 bodies = one For_i
iteration): one ACT copy moves pr [1,80] PSUM -> sc SBUF, then ONE SP
dma_start writes the 320B block to d_out row 0.  This replaces the
per-body HWDGE dma_start (~700ns of SP-queue / shared-HWDGE time per
body -- HWDGE is ONE shared device, so spreading dma_starts across
SP+ACT queues does not parallelize it) with ~9ns/body of SP time and
one convergence point per iteration instead of five.  Rejected
alternatives, measured: SWDGE dma_scatter_add as the partition reducer
(128 RMW descriptors/body) ran ~4955ns/body and even a 16-desc/chunk
scatter ran ~1216ns/body -- SWDGE desc-gen / DMA-side read-modify-write
costs ~39ns+ per descriptor on HW; a [126,25] layout contracting the
xyz components on PE (block-diagonal selector lhsT) ran +135ns/body
paired, because every matmul reloads the selector weights
(--enable-ldw-opt=false) and the chain gains two cross-engine hops.

Engine budget per body (measured issue costs: DVE ~128ns, ACT ~250ns,
Pool tt ~283ns, PE matmul ~117ns): DVE carries nsq/stt + 9/16 of d
(D_PAT), Pool carries 7/16 of d + 11/16 of sq, ACT has sqrt + 5/16 of
sq as Square activations (SQ_PAT 'a') + the chunk copy, PE one matmul
-- the three flexible queues balance at ~330ns.  pred/true are staged
as two dense [128,8,3] tiles so every in-loop operand AP is
unit-stride (the earlier packed [N,8] layout made each d op read
24-of-64-byte strided operand slices, wasting SBUF read beats; dense
staging measured 387/395 vs 396-422 same-window).  Measured
371-395ns/body steady state across this config family's full benches
(the shared TRN2 drifts ~15% run to run; earlier configs: sq-all-Pool
401-432, baseline 479-514).
Swept and rejected: GS=16 fine stage-interleave (447ns -- short stage
phases reintroduce cross-engine rendezvous stalls five times per
group; the long GS=80 phases act as a dependency buffer), UNROLL=160
with CHUNK=80 (434ns), UNROLL=CHUNK=100 (413ns), UNROLL=CHUNK=160
(410ns -- even with distinct tags and aligned patterns: under
stage-major emission the pipeline ramp grows with phase length, so
bigger unrolls cancel their own barrier amortization).  UNROLL=80 is
the sweet spot from both directions.

The bench loop (reps>0) unrolls UNROLL bodies per For_i iteration
(the back-edge runs an all-engine barrier ~1.3us, amortized).  Bodies
are emitted stage-interleaved per chunk (software pipelining) so each
in-order engine queue alternates between independent bodies.

Sharding: batch-parallel, one batch per core (spec hint allows B data-parallel).
"""

import numpy as np

P = 128          # partitions
T = 8            # j = 8*p + t  (p-major; any index bijection works for the sum)
N = 1024
B = 8
NCORES = 8
UNROLL = 80
CHUNK = 80       # bodies per output-write chunk (one tail per iteration)
NCHUNK = UNROLL // CHUNK
CLAMP = 10.0

# which engine computes sq / stt for body slot i in its chunk:
# 'v' = DVE, 'p' = Pool
SQ_PAT = "appappappappappp"
ST_PAT = "vvvvvvvvvvvvvvvv"   # stt must stay 'v': walrus can't lower
                              # accum_out stt on Pool
D_PAT = "vvvvvvvvvppppppp"
# LAYOUT "t8":  j = 8*p + t, tiles [128, 8, 3]; 3-comp sum via DVE reduce.
# LAYOUT "g42": partition = 3*jg + c, j = 25*jg + e, tiles [126, 25];
#   3-comp sum via PE matmul against a constant block-diagonal selector
#   [126, 42] (one DVE op fewer per body; slots 1024..1049 are zero-padded,
#   contributing f(0) = 0 exactly).
LAYOUT = "t8"
GP, GE = 42, 25               # g42: groups, elements per group-slot
TAIL_DELAY = 0                # 1 = emit chunk tails one chunk late
GS = 80                       # emission interleave: bodies per stage-group
# bench output path: "scatter" = SWDGE prep/trigger, "hwdge" = per-body
# dma_start (SP/gpsimd split like the baseline), "block" = one SP dma_start
# per chunk writing the 16 contiguous scalars, "none" = diagnostic only
OUT_MODE = "block"

_cache: dict = {}


def _build_nc(reps=0, prep_only=False):
    """Emit the single-core BIR module (same NEFF runs SPMD on all 8 cores)."""
    from contextlib import ExitStack

    import concourse.bacc as bacc
    import concourse.mybir as mybir
    import concourse.tile as tile
    from concourse._compat import axon_active

    f32 = mybir.dt.float32
    i16 = mybir.dt.int16
    Alu = mybir.AluOpType
    Act = mybir.ActivationFunctionType
    AxX = mybir.AxisListType.X

    nc = bacc.Bacc(
        "TRN2",
        target_bir_lowering=False,
        debug=not axon_active(),
        num_devices=NCORES,
    )

    # One concatenated input: t8 packs [pred*m (3), true*m (3), pad (2)]
    # per j-row; g42 packs [126, 25+25] with partition = 3*jg + component.
    if LAYOUT == "g42":
        d_all = nc.dram_tensor("all_in", [P, 2 * GE], f32,
                               kind="ExternalInput")
    else:
        # pred rows 0:N, true rows N:2N -- staged into two DENSE [128,8,3]
        # tiles so every in-loop operand AP is unit-stride (a packed [N,8]
        # layout made each d op read 24-of-64-byte strided slices).
        d_all = nc.dram_tensor("all_in", [2 * N, 3], f32,
                               kind="ExternalInput")
    # Scatter index table, [16, 1] int16 per chunk: token i sits at
    # [i%16, i//16], so column c holds chunk c's 16 d_out rows.
    nchunk = UNROLL // CHUNK
    assert UNROLL == nchunk * CHUNK
    d_idx = nc.dram_tensor("idx16", [16, nchunk], i16,
                           kind="ExternalInput")
    # Row r, col 0 accumulates body r's scalar; 64-col rows keep the
    # scatter elem_step at 256B as SWDGE requires.  reps=0 uses row 0.
    d_out = nc.dram_tensor("out_acc", [UNROLL, 128], f32, kind="ExternalOutput")

    with tile.TileContext(nc) as tc, ExitStack() as ctx:
        sb = ctx.enter_context(tc.tile_pool(name="sb", bufs=1))
        wpool = ctx.enter_context(tc.tile_pool(name="wpool", bufs=4))
        opool = ctx.enter_context(tc.tile_pool(name="opool", bufs=4))
        pspool = ctx.enter_context(tc.tile_pool(name="pspool", bufs=2,
                                                space="PSUM"))

        dma_sem = nc.alloc_semaphore("swdge_out")

        # ---- ACT table warmup: force the sqrt set load early (overlaps DMA)
        warm = sb.tile([1, 2], f32)
        nc.vector.memset(warm[:], 1.0)
        nc.scalar.activation(warm[:, 1:2], warm[:, 0:1], Act.Sqrt)

        ones = sb.tile([P, 1], f32)
        nc.vector.memset(ones[:], 1.0)

        # ---- zero-fill out_acc once (scatter-add accumulates onto it);
        # the reps=0 path writes its row directly and needs no zeroing.
        if reps:
            # all-zero payload: tile shape only needs to match in byte count
            zrow = sb.tile([P, UNROLL], f32)
            nc.vector.memset(zrow[:], 0.0)
            nc.sync.dma_start(d_out.ap()[:, :], zrow[:])

        # ---- index table load (metadata for scatter desc-gen)
        idxs = sb.tile([16, nchunk], i16)
        nc.sync.dma_start(idxs[:], d_idx.ap())

        # ---- ONE input DMA staging all positions in SBUF.
        if LAYOUT == "g42":
            stg = sb.tile([P, 2 * GE], f32)
            nc.sync.dma_start(stg[:], d_all.ap())
            t_pred = stg[0:3 * GP, 0:GE]
            t_true = stg[0:3 * GP, GE:2 * GE]
            # Block-diagonal selector: SEL[p, g] = 1 iff p // 3 == g, so
            # SEL^T @ sq contracts the 3 components of each group on PE.
            # Host-supplied constant (engine memsets can't write 3-partition
            # slices at arbitrary partition offsets).
            d_sel = nc.dram_tensor("selc", [3 * GP, GP], f32,
                                   kind="ExternalInput")
            sel = sb.tile([3 * GP, GP], f32)
            nc.sync.dma_start(sel[:], d_sel.ap())
        else:
            stg_p = sb.tile([P, T, 3], f32)
            stg_t = sb.tile([P, T, 3], f32)
            nc.sync.dma_start(
                stg_p[:],
                d_all.ap()[0:N].rearrange("(p t) c -> p t c", p=P))
            nc.sync.dma_start(
                stg_t[:],
                d_all.ap()[N:2 * N].rearrange("(p t) c -> p t c", p=P))
            t_pred = stg_p[:]
            t_true = stg_t[:]

        def emit_bodies(rows, chunk_id):
            """Emit len(rows) compute bodies; return state for emit_tail."""
            G = len(rows)
            tag = rows[0] % CHUNK   # pool bufs rotate reused tags
            ps = [opool.tile([P, 1], f32, tag=f"ps{tag + i}", name="ps")
                  for i in range(G)]
            pr = pspool.tile([1, CHUNK], f32, tag=f"pr{chunk_id % 4}",
                             name="pr")
            sc = opool.tile([1, max(128, CHUNK)], f32,
                tag=f"sc{chunk_id % 2}", name="sc")
            tl = {k: [wpool.tile(shp, f32, tag=f"{k}{tag + i}", name=k)
                      for i in range(G)]
                  for k, shp in [("d", [P, T, 3]), ("sq", [P, T, 3]),
                                 ("nsq", [P, T]), ("dm", [P, T]),
                                 ("s", [P, T])]}
            for g0 in range(0, G, GS):
                gr = range(g0, min(g0 + GS, G))
                for i in gr:
                    eng = (nc.vector if D_PAT[i % len(D_PAT)] == "v"
                           else nc.gpsimd)
                    eng.tensor_tensor(tl["d"][i][:], t_pred, t_true,
                                      Alu.subtract)
                for i in gr:
                    w = SQ_PAT[i % len(SQ_PAT)]
                    if w == "a":
                        nc.scalar.activation(tl["sq"][i][:], tl["d"][i][:],
                                             Act.Square)
                    else:
                        eng = nc.vector if w == "v" else nc.gpsimd
                        eng.tensor_tensor(tl["sq"][i][:], tl["d"][i][:],
                                          tl["d"][i][:], Alu.mult)
                for i in gr:
                    nc.vector.tensor_reduce(tl["nsq"][i][:], tl["sq"][i][:],
                                            AxX, Alu.add)
                for i in gr:
                    nc.scalar.activation(tl["dm"][i][:], tl["nsq"][i][:],
                                         Act.Sqrt)
                for i in gr:
                    eng = (nc.vector if ST_PAT[i % len(ST_PAT)] == "v"
                           else nc.gpsimd)
                    eng.scalar_tensor_tensor(
                        tl["s"][i][:], tl["dm"][i][:], CLAMP, tl["dm"][i][:],
                        Alu.min, Alu.add, accum_out=ps[i][:])
            return (rows, chunk_id, G, ps, pr, sc, P)

        def emit_bodies_g42(rows, chunk_id):
            """g42 bodies: d/sq on [126,25]; PE contracts components."""
            G = len(rows)
            tag = rows[0] % CHUNK
            ps = [opool.tile([GP, 1], f32, tag=f"ps{tag + i}", name="ps")
                  for i in range(G)]
            pr = pspool.tile([1, CHUNK], f32, tag=f"pr{chunk_id % 4}",
                             name="pr")
            sc = opool.tile([1, max(128, CHUNK)], f32,
                tag=f"sc{chunk_id % 2}", name="sc")
            nsq = [pspool.tile([GP, GE], f32, tag=f"nsq{(tag + i) % 4}",
                               name="nsq") for i in range(G)]
            tl = {k: [wpool.tile(shp, f32, tag=f"{k}{tag + i}", name=k)
                      for i in range(G)]
                  for k, shp in [("d", [3 * GP, GE]), ("sq", [3 * GP, GE]),
                                 ("dm", [GP, GE]), ("s", [GP, GE])]}
            for i in range(G):
                eng = nc.vector if D_PAT[i % len(D_PAT)] == "v" else nc.gpsimd
                eng.tensor_tensor(tl["d"][i][:], t_pred, t_true,
                                  Alu.subtract)
            for i in range(G):
                eng = nc.vector if SQ_PAT[i % len(SQ_PAT)] == "v" else nc.gpsimd
                eng.tensor_tensor(tl["sq"][i][:], tl["d"][i][:],
                                  tl["d"][i][:], Alu.mult)
            for i in range(G):
                nc.tensor.matmul(nsq[i][:], sel[:], tl["sq"][i][:],
                                 start=True, stop=True)
            for i in range(G):
                nc.scalar.activation(tl["dm"][i][:], nsq[i][:], Act.Sqrt)
            for i in range(G):
                eng = nc.vector if ST_PAT[i % len(ST_PAT)] == "v" else nc.gpsimd
                eng.scalar_tensor_tensor(
                    tl["s"][i][:], tl["dm"][i][:], CLAMP, tl["dm"][i][:],
                    Alu.min, Alu.add, accum_out=ps[i][:])
            return (rows, chunk_id, G, ps, pr, sc, GP)

        def emit_tail(state):
            """Partition-reduce + output write for one chunk of bodies.

            Called one chunk LATE (software pipelining across chunks) so the
            ACT/PE/SP queues aren't stalled at the chunk convergence point.
            """
            rows, chunk_id, G, ps, pr, sc, kp = state
            for i in range(G):
                nc.tensor.matmul(pr[0:1, i:i + 1],
                                 ones[0:kp, :], ps[i][:],
                                 start=True, stop=True)
            if G == CHUNK and OUT_MODE == "scatter":
                nc.scalar.activation(sc[0:1, 0:G], pr[0:1, 0:G], Act.Copy)
                nc.gpsimd.dma_scatter_add(
                    d_out.ap()[:, 0:1],
                    sc[:].rearrange("p (t e) -> p t e", e=1),
                    idxs[:, chunk_id:chunk_id + 1],
                    CHUNK,
                    CHUNK,
                    1,
                    elem_step=64,
                    prepare_only=True,
                    sem=dma_sem,
                )
                nc.gpsimd.trigger_dma(count=None)
            elif G == CHUNK and OUT_MODE == "block":
                nc.scalar.activation(sc[0:1, 0:G], pr[0:1, 0:G], Act.Copy)
                if G <= 128:
                    dst = d_out.ap()[chunk_id:chunk_id + 1, 0:G]
                else:
                    dst = d_out.ap()[0:2, 0:G // 2]   # linear 2-row view
                nc.sync.dma_start(dst, sc[0:1, 0:G])
            elif G == CHUNK and OUT_MODE == "hwdge":
                nc.scalar.activation(sc[0:1, 0:G], pr[0:1, 0:G], Act.Copy)
                for i, row in enumerate(rows):
                    eng = nc.gpsimd if row % 8 in (2, 5, 7) else nc.sync
                    eng.dma_start(d_out.ap()[row:row + 1, 0:1],
                                  sc[0:1, i:i + 1])
            elif G == CHUNK and OUT_MODE == "none":
                nc.scalar.activation(sc[0:1, 0:G], pr[0:1, 0:G], Act.Copy)
            else:
                nc.scalar.activation(sc[0:1, 0:G], pr[0:1, 0:G], Act.Copy)
                nc.sync.dma_start(d_out.ap()[0:1, 0:1], sc[0:1, 0:1])

        emit = emit_bodies_g42 if LAYOUT == "g42" else emit_bodies
        if reps:
            assert reps % UNROLL == 0, f"reps must be a multiple of {UNROLL}"
            with tc.For_i(0, reps // UNROLL, 1):
                pending = None
                for c in range(nchunk):
                    st = emit(list(range(c * CHUNK, (c + 1) * CHUNK)), c)
                    if not TAIL_DELAY:
                        emit_tail(st)
                    else:
                        if pending is not None:
                            emit_tail(pending)
                        pending = st
                if pending is not None:
                    emit_tail(pending)
        else:
            emit_tail(emit([0], 0))

    nc.compile()
    return nc


def _get_nc():
    if "nc" not in _cache:
        _cache["nc"] = _build_nc()
    return _cache["nc"]


def _idx_table():
    nchunk = UNROLL // CHUNK
    idx = np.zeros((16, nchunk), np.int16)
    for c in range(nchunk):
        for i in range(CHUNK):           # token i -> row c*CHUNK + i
            idx[i % 16, c] = c * CHUNK + i
    return idx


def _pack_g42(x):
    """[B, 1024, 3] -> [B, 126, 25]: partition 3*jg + c, col e, j = 25*jg + e."""
    xp = np.zeros((B, GP * GE, 3), np.float32)
    xp[:, :N] = x
    return (xp.reshape(B, GP, GE, 3).transpose(0, 1, 3, 2)
            .reshape(B, 3 * GP, GE))


def make_inmaps(n, ca, c, pred_pos, true_pos, mask):
    m = np.asarray(mask).astype(np.float32)[:, :, None]
    pm = np.asarray(pred_pos, np.float32) * m
    tm = np.asarray(true_pos, np.float32) * m
    if LAYOUT == "g42":
        allc = np.zeros((B, P, 2 * GE), np.float32)
        allc[:, 0:3 * GP, 0:GE] = _pack_g42(pm)
        allc[:, 0:3 * GP, GE:2 * GE] = _pack_g42(tm)
        sel = np.zeros((3 * GP, GP), np.float32)
        sel[np.arange(3 * GP), np.arange(3 * GP) // 3] = 1.0
        extra = {"idx16": _idx_table(), "selc": sel}
    else:
        allc = np.zeros((B, 2 * N, 3), np.float32)
        allc[:, 0:N] = pm
        allc[:, N:2 * N] = tm
        extra = {"idx16": _idx_table()}
    return [{"all_in": allc[b], **extra} for b in range(B)]


def kernel(n, ca, c, pred_pos, true_pos, mask) -> np.ndarray:
    from concourse.bass_utils import run_bass_kernel_spmd

    nc = _get_nc()
    in_maps = make_inmaps(n, ca, c, pred_pos, true_pos, mask)
    res = run_bass_kernel_spmd(nc, in_maps, core_ids=list(range(NCORES)))
    m = np.asarray(mask).astype(np.float64)
    c_b = m.sum(axis=1)                      # per-batch masked-residue count
    total = 0.0
    for b in range(B):
        sheet = float(res.results[b]["out_acc"][0, 0])
        total += c_b[b] * 0.5 * sheet
    return np.asarray(total / (m.sum() + 1e-8), dtype=np.float32)


# revision 58
# speedup vs baseline: 1.2048x; 1.2048x over previous
"""FAPE loss kernel for Trainium2 (Bass/Tile), 8 NeuronCores.

Problem: B=8, N=1024.  reference computes, per batch b:
    R_i, t_i = backbone frames from (n, ca, c)          [N,3,3],[N,3]
    diff[i,j] = || R_i^T (pred_j - t_i) - R_i^T (true_j - t_i) ||
    per_pair  = min(diff,10) + 0.5*(diff - min(diff,10)) = 0.5*(diff + min(diff,10))
    out = sum_b sum_ij m_i m_j per_pair / (sum(m) + 1e-8)

Key identity: both pred and true are expressed in the SAME frame i, so
    R_i^T (pred_j - t_i) - R_i^T (true_j - t_i) = R_i^T d_j,  d_j = pred_j - true_j
and R_i is orthonormal by construction, hence diff[i,j] = ||d_j||
(independent of i) up to the 1e-8 normalize-eps and f32 rounding
(~6e-7 end-to-end vs the jax reference; tolerance is 2e-2).  The O(N^2)
pairwise reduction factorizes exactly:
    sum_ij m_i m_j f(||d_j||) = (sum_i m_i) * (sum_j m_j f(||d_j||))
leaving O(N) device work per batch.  The mask is folded into the packed
inputs on the host (pred_j, true_j both scaled by m_j => masked j gives
d_j = 0 and f(0) = 0, exactly), so the device computes sum_j f(||d_j||).

Per-core body (one batch per core, j = 8*p + t), 6 instrs + output:
    d    = pred - true                        [128,8,3]  DVE/Pool (D_PAT)
    sq   = d * d                              [128,24]   DVE/Pool (SQ_PAT)
    nsq  = reduce_X(sq)                       [128,8]    DVE
    dm   = Sqrt(nsq)                          [128,8]    ACT
    stt  = (dm min 10) + dm, accum_out=ps_b   [128,8]    DVE  (per-partition
           sum fused into the clamp op's accumulate port -> ps_b [128,1])
    mm   = ones[128,1]^T @ ps_b -> pr[0, b]   [1,1]  PE (partition sum into
           body slot b of a chunk-shared [1,CHUNK] PSUM tile)
Output path (OUT_MODE="block", once per CHUNK=80 bodies = one For_i
iteration): one ACT copy moves pr [1,80] PSUM -> sc SBUF, then ONE SP
dma_start writes the 320B block to d_out row 0.  This replaces the
per-body HWDGE dma_start (~700ns of SP-queue / shared-HWDGE time per
body -- HWDGE is ONE shared device, so spreading dma_starts across
SP+ACT queues does not parallelize it) with ~9ns/body of SP time and
one convergence point per iteration instead of five.  Rejected
alternatives, measured: SWDGE dma_scatter_add as the partition reducer
(128 RMW descriptors/body) ran ~4955ns/body and even a 16-desc/chunk
scatter ran ~1216ns/body -- SWDGE desc-gen / DMA-side read-modify-write
costs ~39ns+ per descriptor on HW; a [126,25] layout contracting the
xyz components on PE (block-diagonal selector lhsT) ran +135ns/body
paired, because every matmul reloads the selector weights
(--enable-ldw-opt=false) and the chain gains two cross-engine hops.

Engine budget per body (measured issue costs: DVE ~128ns, ACT ~250ns,
Pool tt ~283ns, PE matmul ~117ns): DVE carries nsq/stt + 9/16 of d
(D_PAT), Pool carries 7/16 of d + 11/16 of sq, ACT has sqrt + 5/16 of
sq as Square activations (SQ_PAT 'a') + the chunk copy, PE one matmul
-- the three flexible queues balance at ~330ns.  pred/true are staged
as two dense [128,8,3] tiles so every in-loop operand AP is
unit-stride (the earlier packed [N,8] layout made each d op read
24-of-64-byte strided operand slices, wasting SBUF read beats; dense
staging measured 387/395 vs 396-422 same-window).  Measured
371-395ns/body steady state across this config family's full benches
(the shared TRN2 drifts ~15% run to run; earlier configs: sq-all-Pool
401-432, baseline 479-514).
Swept and rejected: GS=16 fine stage-interleave (447ns -- short stage
phases reintroduce cross-engine rendezvous stalls five times per
group; the long GS=80 phases act as a dependency buffer), UNROLL=160
with CHUNK=80 (434ns), UNROLL=CHUNK=100 (413ns), UNROLL=CHUNK=160
(410ns -- even with distinct tags and aligned patterns: under
stage-major emission the pipeline ramp grows with phase length, so
bigger unrolls cancel their own barrier amortization).  UNROLL=80 is
the sweet spot from both directions.

The bench loop (reps>0) unrolls UNROLL bodies per For_i iteration
(the back-edge runs an all-engine barrier ~1.3us, amortized).  Bodies
are emitted stage-interleaved per chunk (software pipelining) so each
in-order engine queue alternates between independent bodies.

Sharding: batch-parallel, one batch per core (spec hint allows B data-parallel).
"""

import numpy as np

P = 128          # partitions
T = 8            # j = 8*p + t  (p-major; any index bijection works for the sum)
N = 1024
B = 8
NCORES = 8
UNROLL = 80
CHUNK = 80       # bodies per output-write chunk (one tail per iteration)
NCHUNK = UNROLL // CHUNK
CLAMP = 10.0

# which engine computes sq / stt for body slot i in its chunk:
# 'v' = DVE, 'p' = Pool
SQ_PAT = "appappappappappp"
ST_PAT = "vvvvvvvvvvvvvvvv"   # stt must stay 'v': walrus can't lower
                              # accum_out stt on Pool
D_PAT = "vvvvvvvvvppppppp"
# LAYOUT "t8":  j = 8*p + t, tiles [128, 8, 3]; 3-comp sum via DVE reduce.
# LAYOUT "g42": partition = 3*jg + c, j = 25*jg + e, tiles [126, 25];
#   3-comp sum via PE matmul against a constant block-diagonal selector
#   [126, 42] (one DVE op fewer per body; slots 1024..1049 are zero-padded,
#   contributing f(0) = 0 exactly).
LAYOUT = "t8"
GP, GE = 42, 25               # g42: groups, elements per group-slot
TAIL_DELAY = 0                # 1 = emit chunk tails one chunk late
GS = 80                       # emission interleave: bodies per stage-group
QORDER = 1                    # supply-ordered queue surgery (see below)
# bench output path: "scatter" = SWDGE prep/trigger, "hwdge" = per-body
# dma_start (SP/gpsimd split like the baseline), "block" = one SP dma_start
# per chunk writing the 16 contiguous scalars, "none" = diagnostic only
OUT_MODE = "block"

_cache: dict = {}


def _build_nc(reps=0, prep_only=False):
    """Emit the single-core BIR module (same NEFF runs SPMD on all 8 cores)."""
    from contextlib import ExitStack

    import concourse.bacc as bacc
    import concourse.mybir as mybir
    import concourse.tile as tile
    from concourse._compat import axon_active

    f32 = mybir.dt.float32
    i16 = mybir.dt.int16
    Alu = mybir.AluOpType
    Act = mybir.ActivationFunctionType
    AxX = mybir.AxisListType.X

    nc = bacc.Bacc(
        "TRN2",
        target_bir_lowering=False,
        debug=not axon_active(),
        num_devices=NCORES,
    )

    # One concatenated input: t8 packs [pred*m (3), true*m (3), pad (2)]
    # per j-row; g42 packs [126, 25+25] with partition = 3*jg + component.
    if LAYOUT == "g42":
        d_all = nc.dram_tensor("all_in", [P, 2 * GE], f32,
                               kind="ExternalInput")
    else:
        # pred rows 0:N, true rows N:2N -- staged into two DENSE [128,8,3]
        # tiles so every in-loop operand AP is unit-stride (a packed [N,8]
        # layout made each d op read 24-of-64-byte strided slices).
        d_all = nc.dram_tensor("all_in", [2 * N, 3], f32,
                               kind="ExternalInput")
    # Scatter index table, [16, 1] int16 per chunk: token i sits at
    # [i%16, i//16], so column c holds chunk c's 16 d_out rows.
    nchunk = UNROLL // CHUNK
    assert UNROLL == nchunk * CHUNK
    d_idx = nc.dram_tensor("idx16", [16, nchunk], i16,
                           kind="ExternalInput")
    # Row r, col 0 accumulates body r's scalar; 64-col rows keep the
    # scatter elem_step at 256B as SWDGE requires.  reps=0 uses row 0.
    d_out = nc.dram_tensor("out_acc", [UNROLL, 128], f32, kind="ExternalOutput")

    with tile.TileContext(nc) as tc, ExitStack() as ctx:
        sb = ctx.enter_context(tc.tile_pool(name="sb", bufs=1))
        wpool = ctx.enter_context(tc.tile_pool(name="wpool", bufs=4))
        opool = ctx.enter_context(tc.tile_pool(name="opool", bufs=4))
        pspool = ctx.enter_context(tc.tile_pool(name="pspool", bufs=2,
                                                space="PSUM"))

        dma_sem = nc.alloc_semaphore("swdge_out")

        # ---- ACT table warmup: force the sqrt set load early (overlaps DMA)
        warm = sb.tile([1, 2], f32)
        nc.vector.memset(warm[:], 1.0)
        nc.scalar.activation(warm[:, 1:2], warm[:, 0:1], Act.Sqrt)

        ones = sb.tile([P, 1], f32)
        nc.vector.memset(ones[:], 1.0)

        # ---- zero-fill out_acc once (scatter-add accumulates onto it);
        # the reps=0 path writes its row directly and needs no zeroing.
        if reps:
            # all-zero payload: tile shape only needs to match in byte count
            zrow = sb.tile([P, UNROLL], f32)
            nc.vector.memset(zrow[:], 0.0)
            nc.sync.dma_start(d_out.ap()[:, :], zrow[:])

        # ---- index table load (metadata for scatter desc-gen)
        idxs = sb.tile([16, nchunk], i16)
        nc.sync.dma_start(idxs[:], d_idx.ap())

        # ---- ONE input DMA staging all positions in SBUF.
        if LAYOUT == "g42":
            stg = sb.tile([P, 2 * GE], f32)
            nc.sync.dma_start(stg[:], d_all.ap())
            t_pred = stg[0:3 * GP, 0:GE]
            t_true = stg[0:3 * GP, GE:2 * GE]
            # Block-diagonal selector: SEL[p, g] = 1 iff p // 3 == g, so
            # SEL^T @ sq contracts the 3 components of each group on PE.
            # Host-supplied constant (engine memsets can't write 3-partition
            # slices at arbitrary partition offsets).
            d_sel = nc.dram_tensor("selc", [3 * GP, GP], f32,
                                   kind="ExternalInput")
            sel = sb.tile([3 * GP, GP], f32)
            nc.sync.dma_start(sel[:], d_sel.ap())
        else:
            stg_p = sb.tile([P, T, 3], f32)
            stg_t = sb.tile([P, T, 3], f32)
            nc.sync.dma_start(
                stg_p[:],
                d_all.ap()[0:N].rearrange("(p t) c -> p t c", p=P))
            nc.sync.dma_start(
                stg_t[:],
                d_all.ap()[N:2 * N].rearrange("(p t) c -> p t c", p=P))
            t_pred = stg_p[:]
            t_true = stg_t[:]

        def emit_bodies(rows, chunk_id):
            """Emit len(rows) compute bodies; return state for emit_tail."""
            G = len(rows)
            tag = rows[0] % CHUNK   # pool bufs rotate reused tags
            ps = [opool.tile([P, 1], f32, tag=f"ps{tag + i}", name="ps")
                  for i in range(G)]
            pr = pspool.tile([1, CHUNK], f32, tag=f"pr{chunk_id % 4}",
                             name="pr")
            sc = opool.tile([1, max(128, CHUNK)], f32,
                tag=f"sc{chunk_id % 2}", name="sc")
            tl = {k: [wpool.tile(shp, f32, tag=f"{k}{tag + i}", name=k)
                      for i in range(G)]
                  for k, shp in [("d", [P, T, 3]), ("sq", [P, T, 3]),
                                 ("nsq", [P, T]), ("dm", [P, T]),
                                 ("s", [P, T])]}
            def emit_d(i):
                eng = (nc.vector if D_PAT[i % len(D_PAT)] == "v"
                       else nc.gpsimd)
                eng.tensor_tensor(tl["d"][i][:], t_pred, t_true,
                                  Alu.subtract)

            def emit_sq(i):
                w = SQ_PAT[i % len(SQ_PAT)]
                if w == "a":
                    nc.scalar.activation(tl["sq"][i][:], tl["d"][i][:],
                                         Act.Square)
                else:
                    eng = nc.vector if w == "v" else nc.gpsimd
                    eng.tensor_tensor(tl["sq"][i][:], tl["d"][i][:],
                                      tl["d"][i][:], Alu.mult)

            if QORDER:
                # Queue-order surgery: Pool's stage-major order
                # [all d][all sq] makes DVE's nsq phase wait ~4us for
                # Pool to clear its d-phase.  Instead: DVE-fed Pool sq's
                # first (ready as soon as DVE's d-phase streams), then
                # Pool's own bodies as (d, sq) pairs; downstream stages
                # consume in matching supply order.
                vd = [i for i in range(G)
                      if D_PAT[i % len(D_PAT)] == "v"]
                pd = [i for i in range(G)
                      if D_PAT[i % len(D_PAT)] != "v"]
                order = ([i for i in vd if SQ_PAT[i % len(SQ_PAT)] == "a"]
                         + [i for i in vd if SQ_PAT[i % len(SQ_PAT)] != "a"]
                         + pd)
                for i in vd:
                    emit_d(i)
                for i in vd:
                    emit_sq(i)
                for i in pd:
                    emit_d(i)
                    emit_sq(i)
                for i in order:
                    nc.vector.tensor_reduce(tl["nsq"][i][:], tl["sq"][i][:],
                                            AxX, Alu.add)
                for i in order:
                    nc.scalar.activation(tl["dm"][i][:], tl["nsq"][i][:],
                                         Act.Sqrt)
                for i in order:
                    eng = (nc.vector if ST_PAT[i % len(ST_PAT)] == "v"
                           else nc.gpsimd)
                    eng.scalar_tensor_tensor(
                        tl["s"][i][:], tl["dm"][i][:], CLAMP, tl["dm"][i][:],
                        Alu.min, Alu.add, accum_out=ps[i][:])
                return (rows, chunk_id, G, ps, pr, sc, P)

            for g0 in range(0, G, GS):
                gr = range(g0, min(g0 + GS, G))
                for i in gr:
                    emit_d(i)
                for i in gr:
                    emit_sq(i)
                for i in gr:
                    nc.vector.tensor_reduce(tl["nsq"][i][:], tl["sq"][i][:],
                                            AxX, Alu.add)
                for i in gr:
                    nc.scalar.activation(tl["dm"][i][:], tl["nsq"][i][:],
                                         Act.Sqrt)
                for i in gr:
                    eng = (nc.vector if ST_PAT[i % len(ST_PAT)] == "v"
                           else nc.gpsimd)
                    eng.scalar_tensor_tensor(
                        tl["s"][i][:], tl["dm"][i][:], CLAMP, tl["dm"][i][:],
                        Alu.min, Alu.add, accum_out=ps[i][:])
            return (rows, chunk_id, G, ps, pr, sc, P)

        def emit_bodies_g42(rows, chunk_id):
            """g42 bodies: d/sq on [126,25]; PE contracts components."""
            G = len(rows)
            tag = rows[0] % CHUNK
            ps = [opool.tile([GP, 1], f32, tag=f"ps{tag + i}", name="ps")
                  for i in range(G)]
            pr = pspool.tile([1, CHUNK], f32, tag=f"pr{chunk_id % 4}",
                             name="pr")
            sc = opool.tile([1, max(128, CHUNK)], f32,
                tag=f"sc{chunk_id % 2}", name="sc")
            nsq = [pspool.tile([GP, GE], f32, tag=f"nsq{(tag + i) % 4}",
                               name="nsq") for i in range(G)]
            tl = {k: [wpool.tile(shp, f32, tag=f"{k}{tag + i}", name=k)
                      for i in range(G)]
                  for k, shp in [("d", [3 * GP, GE]), ("sq", [3 * GP, GE]),
                                 ("dm", [GP, GE]), ("s", [GP, GE])]}
            for i in range(G):
                eng = nc.vector if D_PAT[i % len(D_PAT)] == "v" else nc.gpsimd
                eng.tensor_tensor(tl["d"][i][:], t_pred, t_true,
                                  Alu.subtract)
            for i in range(G):
                eng = nc.vector if SQ_PAT[i % len(SQ_PAT)] == "v" else nc.gpsimd
                eng.tensor_tensor(tl["sq"][i][:], tl["d"][i][:],
                                  tl["d"][i][:], Alu.mult)
            for i in range(G):
                nc.tensor.matmul(nsq[i][:], sel[:], tl["sq"][i][:],
                                 start=True, stop=True)
            for i in range(G):
                nc.scalar.activation(tl["dm"][i][:], nsq[i][:], Act.Sqrt)
            for i in range(G):
                eng = nc.vector if ST_PAT[i % len(ST_PAT)] == "v" else nc.gpsimd
                eng.scalar_tensor_tensor(
                    tl["s"][i][:], tl["dm"][i][:], CLAMP, tl["dm"][i][:],
                    Alu.min, Alu.add, accum_out=ps[i][:])
            return (rows, chunk_id, G, ps, pr, sc, GP)

        def emit_tail(state):
            """Partition-reduce + output write for one chunk of bodies.

            Called one chunk LATE (software pipelining across chunks) so the
            ACT/PE/SP queues aren't stalled at the chunk convergence point.
            """
            rows, chunk_id, G, ps, pr, sc, kp = state
            for i in range(G):
                nc.tensor.matmul(pr[0:1, i:i + 1],
                                 ones[0:kp, :], ps[i][:],
                                 start=True, stop=True)
            if G == CHUNK and OUT_MODE == "scatter":
                nc.scalar.activation(sc[0:1, 0:G], pr[0:1, 0:G], Act.Copy)
                nc.gpsimd.dma_scatter_add(
                    d_out.ap()[:, 0:1],
                    sc[:].rearrange("p (t e) -> p t e", e=1),
                    idxs[:, chunk_id:chunk_id + 1],
                    CHUNK,
                    CHUNK,
                    1,
                    elem_step=64,
                    prepare_only=True,
                    sem=dma_sem,
                )
                nc.gpsimd.trigger_dma(count=None)
            elif G == CHUNK and OUT_MODE == "block":
                nc.scalar.activation(sc[0:1, 0:G], pr[0:1, 0:G], Act.Copy)
                if G <= 128:
                    dst = d_out.ap()[chunk_id:chunk_id + 1, 0:G]
                else:
                    dst = d_out.ap()[0:2, 0:G // 2]   # linear 2-row view
                nc.sync.dma_start(dst, sc[0:1, 0:G])
            elif G == CHUNK and OUT_MODE == "hwdge":
                nc.scalar.activation(sc[0:1, 0:G], pr[0:1, 0:G], Act.Copy)
                for i, row in enumerate(rows):
                    eng = nc.gpsimd if row % 8 in (2, 5, 7) else nc.sync
                    eng.dma_start(d_out.ap()[row:row + 1, 0:1],
                                  sc[0:1, i:i + 1])
            elif G == CHUNK and OUT_MODE == "none":
                nc.scalar.activation(sc[0:1, 0:G], pr[0:1, 0:G], Act.Copy)
            else:
                nc.scalar.activation(sc[0:1, 0:G], pr[0:1, 0:G], Act.Copy)
                nc.sync.dma_start(d_out.ap()[0:1, 0:1], sc[0:1, 0:1])

        emit = emit_bodies_g42 if LAYOUT == "g42" else emit_bodies
        if reps:
            assert reps % UNROLL == 0, f"reps must be a multiple of {UNROLL}"
            with tc.For_i(0, reps // UNROLL, 1):
                pending = None
                for c in range(nchunk):
                    st = emit(list(range(c * CHUNK, (c + 1) * CHUNK)), c)
                    if not TAIL_DELAY:
                        emit_tail(st)
                    else:
                        if pending is not None:
                            emit_tail(pending)
                        pending = st
                if pending is not None:
                    emit_tail(pending)
        else:
            emit_tail(emit([0], 0))

    nc.compile()
    return nc


def _get_nc():
    if "nc" not in _cache:
        _cache["nc"] = _build_nc()
    return _cache["nc"]


def _idx_table():
    nchunk = UNROLL // CHUNK
    idx = np.zeros((16, nchunk), np.int16)
    for c in range(nchunk):
        for i in range(CHUNK):           # token i -> row c*CHUNK + i
            idx[i % 16, c] = c * CHUNK + i
    return idx


def _pack_g42(x):
    """[B, 1024, 3] -> [B, 126, 25]: partition 3*jg + c, col e, j = 25*jg + e."""
    xp = np.zeros((B, GP * GE, 3), np.float32)
    xp[:, :N] = x
    return (xp.reshape(B, GP, GE, 3).transpose(0, 1, 3, 2)
            .reshape(B, 3 * GP, GE))


def make_inmaps(n, ca, c, pred_pos, true_pos, mask):
    m = np.asarray(mask).astype(np.float32)[:, :, None]
    pm = np.asarray(pred_pos, np.float32) * m
    tm = np.asarray(true_pos, np.float32) * m
    if LAYOUT == "g42":
        allc = np.zeros((B, P, 2 * GE), np.float32)
        allc[:, 0:3 * GP, 0:GE] = _pack_g42(pm)
        allc[:, 0:3 * GP, GE:2 * GE] = _pack_g42(tm)
        sel = np.zeros((3 * GP, GP), np.float32)
        sel[np.arange(3 * GP), np.arange(3 * GP) // 3] = 1.0
        extra = {"idx16": _idx_table(), "selc": sel}
    else:
        allc = np.zeros((B, 2 * N, 3), np.float32)
        allc[:, 0:N] = pm
        allc[:, N:2 * N] = tm
        extra = {"idx16": _idx_table()}
    return [{"all_in": allc[b], **extra} for b in range(B)]


def kernel(n, ca, c, pred_pos, true_pos, mask) -> np.ndarray:
    from concourse.bass_utils import run_bass_kernel_spmd

    nc = _get_nc()
    in_maps = make_inmaps(n, ca, c, pred_pos, true_pos, mask)
    res = run_bass_kernel_spmd(nc, in_maps, core_ids=list(range(NCORES)))
    m = np.asarray(mask).astype(np.float64)
    c_b = m.sum(axis=1)                      # per-batch masked-residue count
    total = 0.0
    for b in range(B):
        sheet = float(res.results[b]["out_acc"][0, 0])
        total += c_b[b] * 0.5 * sheet
    return np.asarray(total / (m.sum() + 1e-8), dtype=np.float32)
